# revision 8
# baseline (speedup 1.0000x reference)
"""Trainium2 Bass kernel: 3D bilateral filter (5x5x5, replicate pad).

Reference math (the wd normalization cancels in the final ratio):
    out(v) = sum_k g_k e_k p_k / sum_k g_k e_k,   e_k = exp(-a (p_k - c)^2)
with a = 1/(2*0.8^2), g the separable 5x5x5 gaussian, p_k the 125
replicate-padded shifted neighbours and c the center voxel.

Substituting q_k = p_k - c gives out = c + (sum g e q) / (sum g e), which
needs only the center-subtracted patches on device.  e_k is computed in a
single ACT pass as Derivative_Erf(sqrt(a) q) = 2/sqrt(pi) exp(-a q^2) -- the
2/sqrt(pi) factor is common to numerator and denominator and cancels.

Device strategy (per core, data-parallel over 144 (c,d) planes, 18 each):
  - host ships Q[125, 41472] bf16 (im2col minus center), cen bf16, and the
    gaussian one-hot lhsT zg (g_k at column 64)
  - per ~3072-col super-block: one im2col DMA, one ACT pass E = derf(sqrt(a) Q)
    and one DVE pass T = E*Q (both bf16; TT runs in 2x_1p mode)
  - PE reduces taps via one-hot matmuls: 512-voxel block b -> one PSUM row
    (num and den in separate banks); g_k rides the lhsT so no bias pass
  - 81 blocks split into 4 accumulation chains on DISJOINT bank pairs
    (chain i: num bank 2i, den bank 2i+1), so each chain's epilogue overlaps
    the later chains' matmuls with no write-after-read PE stall
  - epilogue per chain: r = 1/den (fast approx), o = bf16(num*r) + cen, DMA out
  - warm-up dummy matmuls keep the PE p-state ramping during pipeline fill

The PE never produces an input of the per-block pipeline (the baseline's
center-broadcast made the in-order PE both head and tail of every block's
dependency chain, serializing the whole kernel).
"""

import math
from contextlib import ExitStack

import numpy as np

import concourse.bass as bass
import concourse.mybir as mybir
import concourse.tile as tile
from concourse import bacc
from concourse.bass_utils import run_bass_kernel_spmd

F32 = mybir.dt.float32
BF16 = mybir.dt.float16  # fp16: same 2-byte engine speed as bf16, 8x finer mantissa
DERF = mybir.ActivationFunctionType.Derivative_Erf

SIGMA = 0.8
A_ = 1.0 / (2.0 * SIGMA * SIGMA)  # 0.78125
SQA = math.sqrt(A_)
KS = 5
NTAP = KS * KS * KS  # 125
NCORES = 8
C_, D_, H_, W_ = 3, 48, 48, 48
PPC = (C_ * D_) // NCORES  # 18 planes per core
PLANE_V = H_ * W_  # 2304
VOX = PPC * PLANE_V  # 41472 voxels per core
V = 512  # PSUM block: one bank num + one bank den
NBLK = VOX // V  # 81
CHAIN_START = (0, 21, 42, 63)  # block ranges of the 4 chains
CHAIN_END = (21, 42, 63, NBLK)
# super-blocks: DMA/ACT/DVE instruction granularity; small head for fast fill
SUPS = (1536, 1536) + (3072,) * 12 + (1536,)
N_WARMUP = 24  # PE p-state warm-up dummies during pipeline fill


def _gauss() -> np.ndarray:
    """The normalized separable 5x5x5 gaussian, float32 [125]."""
    sig = [0.3 * ((k - 1) * 0.5 - 1.0) + 0.8 for k in (KS, KS, KS)]
    grids = np.meshgrid(*[np.arange(k) for k in (KS, KS, KS)], indexing="ij")
    ker = np.ones((KS, KS, KS), dtype=np.float64)
    for k, s, m in zip((KS, KS, KS), sig, grids):
        mean = (k - 1) / 2.0
        ker = ker * np.exp(-((m - mean) ** 2) / (2.0 * s * s))
    ker = ker / ker.sum()
    return ker.astype(np.float32).reshape(-1)


def _chain_of(b: int) -> int:
    for ci in range(4):
        if b < CHAIN_END[ci]:
            return ci
    raise AssertionError(b)


def _kernel_body(ctx, tc, q, cen, zg, outp, repeat: int = 1, loop: int = 1):
    nc = tc.nc

    consts = ctx.enter_context(tc.tile_pool(name="consts", bufs=1))
    q_pool = ctx.enter_context(tc.tile_pool(name="q", bufs=4))
    e_pool = ctx.enter_context(tc.tile_pool(name="e", bufs=3))
    t_pool = ctx.enter_context(tc.tile_pool(name="t", bufs=3))
    epi_pool = ctx.enter_context(tc.tile_pool(name="epi", bufs=2))
    acc_pool = ctx.enter_context(tc.tile_pool(name="acc", bufs=1, space="PSUM"))

    zg_t = consts.tile([NTAP, 128], BF16)
    nc.sync.dma_start(zg_t[:], zg[:])
    cen_ts = []
    for ci in range(4):
        b0, b1 = CHAIN_START[ci], CHAIN_END[ci]
        cen_c = consts.tile([b1 - b0, V], BF16, tag=f"cen{ci}")
        nc.sync.dma_start(
            cen_c[:], cen[b0 * V : b1 * V].rearrange("(b v) -> b v", v=V)
        )
        cen_ts.append(cen_c)

    # chain ci: num -> bank 2ci (col 1024*ci), den -> bank 2ci+1; block b of
    # the chain accumulates into row b - CHAIN_START[ci]
    acc = acc_pool.tile([128, 4096], F32)

    # PE matmuls only support a single sync-wait: consume zg's DMA semaphore
    # with a throwaway matmul; follow with free-running dummies that keep the
    # PE busy through its p-state ramp while the first DMA/ACT fill runs.
    for _ in range(N_WARMUP):
        nc.tensor.matmul(
            acc[0:64, 0:128], zg_t[:, 0:64], zg_t[:, 0:128],
            start=True, stop=True, skip_group_check=True,
        )

    def epilogue(ci):
        nrows = CHAIN_END[ci] - CHAIN_START[ci]
        rows = slice(0, nrows)
        nco = 1024 * ci
        r_t = epi_pool.tile([21, V], F32, tag="r")
        nc.vector.reciprocal_approx_fast(
            out=r_t[rows, :], in_=acc[rows, nco + 512 : nco + 1024]
        )
        m_t = epi_pool.tile([21, V], BF16, tag="m")
        nc.vector.tensor_mul(m_t[rows, :], acc[rows, nco : nco + 512], r_t[rows, :])
        o_t = epi_pool.tile([21, V], BF16, tag="o")
        nc.vector.tensor_add(o_t[rows, :], m_t[rows, :], cen_ts[ci][rows, :])
        nc.sync.dma_start(
            outp[CHAIN_START[ci] * V : CHAIN_END[ci] * V].rearrange(
                "(b v) -> b v", v=V
            ),
            o_t[rows, :],
        )

    def rep_body():
        c0 = 0
        for w in SUPS:
            q_t = q_pool.tile([NTAP, max(SUPS)], BF16)
            nc.sync.dma_start(q_t[:, 0:w], q[:, c0 : c0 + w])
            e_t = e_pool.tile([NTAP, max(SUPS)], BF16)
            nc.scalar.activation(e_t[:, 0:w], q_t[:, 0:w], DERF, scale=SQA)
            t_t = t_pool.tile([NTAP, max(SUPS)], BF16)
            nc.vector.tensor_mul(t_t[:, 0:w], e_t[:, 0:w], q_t[:, 0:w])

            blk0 = c0 // V
            # den matmuls first (they only need the ACT output), then num
            for src_t, bank_off in ((e_t, 512), (t_t, 0)):
                for bi in range(w // V):
                    b = blk0 + bi
                    ci = _chain_of(b)
                    row = b - CHAIN_START[ci]
                    nco = 1024 * ci + bank_off
                    nc.tensor.matmul(
                        acc[0:64, nco : nco + 512],
                        zg_t[:, 64 - row : 128 - row],
                        src_t[:, bi * V : (bi + 1) * V],
                        start=b == CHAIN_START[ci],
                        stop=b == CHAIN_END[ci] - 1,
                    )
                    if bank_off == 0 and b == CHAIN_END[ci] - 1:
                        epilogue(ci)
            c0 += w

    if loop > 1:
        with tc.For_i(0, loop):
            rep_body()
    else:
        for _rep in range(repeat):
            rep_body()


def build_program(repeat: int = 1, loop: int = 1) -> bass.Bass:
    nc = bacc.Bacc("TRN2", target_bir_lowering=False, debug=False)
    q = nc.declare_dram_parameter("q", [NTAP, VOX], BF16, isOutput=False)
    cen = nc.declare_dram_parameter("cen", [VOX], BF16, isOutput=False)
    zg = nc.declare_dram_parameter("zg", [NTAP, 128], BF16, isOutput=False)
    outp = nc.declare_dram_parameter("out", [VOX], BF16, isOutput=True)
    with tile.TileContext(nc) as tc, ExitStack() as ctx:
        _kernel_body(ctx, tc, q, cen, zg, outp, repeat=repeat, loop=loop)
    nc.compile()
    return nc


def build_host_inputs(x: np.ndarray) -> list[dict[str, np.ndarray]]:
    """x: [1, 3, 48, 48, 48] float32 -> per-core in_maps."""
    x = np.asarray(x).reshape(C_, D_, H_, W_).astype(np.float32)
    xp = np.pad(x, ((0, 0), (2, 2), (2, 2), (2, 2)), mode="edge")  # [3,52,52,52]
    zg = np.zeros((NTAP, 128), dtype=np.float32)
    zg[:, 64] = _gauss()
    zg = zg.astype(np.float16)
    in_maps = []
    for m in range(NCORES):
        qm = np.empty((NTAP, VOX), dtype=np.float32)
        cenm = np.empty(VOX, dtype=np.float32)
        for o in range(PPC):
            p = m * PPC + o
            c, d = divmod(p, D_)
            center = x[c, d]  # [48, 48]
            cenm[o * PLANE_V : (o + 1) * PLANE_V] = center.reshape(-1)
            for i in range(KS):
                win = np.lib.stride_tricks.sliding_window_view(
                    xp[c, d + i], (H_, W_)
                )  # [5, 5, 48, 48]
                qm[i * 25 : (i + 1) * 25, o * PLANE_V : (o + 1) * PLANE_V] = (
                    win - center
                ).reshape(25, PLANE_V)
        in_maps.append(
            {
                "q": qm.astype(np.float16),
                "cen": cenm.astype(np.float16),
                "zg": zg,
            }
        )
    return in_maps


_PROGRAM: bass.Bass | None = None


def _get_program() -> bass.Bass:
    global _PROGRAM
    if _PROGRAM is None:
        _PROGRAM = build_program()
    return _PROGRAM


def kernel(x: np.ndarray) -> np.ndarray:
    nc = _get_program()
    in_maps = build_host_inputs(x)
    res = run_bass_kernel_spmd(nc, in_maps, list(range(NCORES)))
    planes = np.concatenate(
        [
            np.asarray(res.results[m]["out"]).astype(np.float32).reshape(PPC, H_, W_)
            for m in range(NCORES)
        ],
        axis=0,
    )  # [144, 48, 48]
    return planes.reshape(1, C_, D_, H_, W_).astype(np.float32)


# revision 10
# speedup vs baseline: 1.9883x; 1.9883x over previous
"""Trainium2 Bass kernel: 3D bilateral filter (5x5x5, replicate pad).

Reference math (the wd normalization cancels in the final ratio):
    out(v) = sum_k g_k e_k p_k / sum_k g_k e_k,   e_k = exp(-a (p_k - c)^2)
with a = 1/(2*0.8^2), g the separable 5x5x5 gaussian, p_k the 125
replicate-padded shifted neighbours and c the center voxel.

Substituting q_k = p_k - c gives out = c + (sum g e q) / (sum g e), which
needs only the center-subtracted patches on device.  e_k is computed in a
single ACT pass as Derivative_Erf(sqrt(a) q) = 2/sqrt(pi) exp(-a q^2) -- the
2/sqrt(pi) factor is common to numerator and denominator and cancels.

Device strategy (per core, data-parallel over 144 (c,d) planes, 18 each):
  - host ships Q[125, 41472] bf16 (im2col minus center), cen bf16, and the
    gaussian one-hot lhsT zg (g_k at column 64)
  - per ~3072-col super-block: one im2col DMA, one ACT pass E = derf(sqrt(a) Q)
    and one DVE pass T = E*Q (both bf16; TT runs in 2x_1p mode)
  - PE reduces taps via one-hot matmuls: 512-voxel block b -> one PSUM row
    (num and den in separate banks); g_k rides the lhsT so no bias pass
  - 81 blocks split into 4 accumulation chains on DISJOINT bank pairs
    (chain i: num bank 2i, den bank 2i+1), so each chain's epilogue overlaps
    the later chains' matmuls with no write-after-read PE stall
  - epilogue per chain: r = 1/den (fast approx), o = bf16(num*r) + cen, DMA out
  - warm-up dummy matmuls keep the PE p-state ramping during pipeline fill

The PE never produces an input of the per-block pipeline (the baseline's
center-broadcast made the in-order PE both head and tail of every block's
dependency chain, serializing the whole kernel).
"""

import math
from contextlib import ExitStack

import numpy as np

import concourse.bass as bass
import concourse.mybir as mybir
import concourse.tile as tile
from concourse import bacc
from concourse.bass_utils import run_bass_kernel_spmd

F32 = mybir.dt.float32
BF16 = mybir.dt.float16  # fp16: same 2-byte engine speed as bf16, 8x finer mantissa
DERF = mybir.ActivationFunctionType.Derivative_Erf

SIGMA = 0.8
A_ = 1.0 / (2.0 * SIGMA * SIGMA)  # 0.78125
SQA = math.sqrt(A_)
KS = 5
NTAP = KS * KS * KS  # 125
NCORES = 8
C_, D_, H_, W_ = 3, 48, 48, 48
PPC = (C_ * D_) // NCORES  # 18 planes per core
PLANE_V = H_ * W_  # 2304
VOX = PPC * PLANE_V  # 41472 voxels per core
V = 512  # PSUM block: one bank num + one bank den
NBLK = VOX // V  # 81
CHAIN_START = (0, 21, 42, 63)  # block ranges of the 4 chains
CHAIN_END = (21, 42, 63, NBLK)
# super-blocks: DMA/ACT/DVE instruction granularity; small head for fast fill
SUPS = (1536, 1536) + (3072,) * 12 + (1536,)
N_WARMUP = 24  # PE p-state warm-up dummies during pipeline fill


def _gauss() -> np.ndarray:
    """The normalized separable 5x5x5 gaussian, float32 [125]."""
    sig = [0.3 * ((k - 1) * 0.5 - 1.0) + 0.8 for k in (KS, KS, KS)]
    grids = np.meshgrid(*[np.arange(k) for k in (KS, KS, KS)], indexing="ij")
    ker = np.ones((KS, KS, KS), dtype=np.float64)
    for k, s, m in zip((KS, KS, KS), sig, grids):
        mean = (k - 1) / 2.0
        ker = ker * np.exp(-((m - mean) ** 2) / (2.0 * s * s))
    ker = ker / ker.sum()
    return ker.astype(np.float32).reshape(-1)


def _chain_of(b: int) -> int:
    for ci in range(4):
        if b < CHAIN_END[ci]:
            return ci
    raise AssertionError(b)


def _kernel_body(ctx, tc, q, cen, zg, outp, repeat: int = 1, loop: int = 1):
    nc = tc.nc

    consts = ctx.enter_context(tc.tile_pool(name="consts", bufs=1))
    q_pool = ctx.enter_context(tc.tile_pool(name="q", bufs=4))
    e_pool = ctx.enter_context(tc.tile_pool(name="e", bufs=3))
    t_pool = ctx.enter_context(tc.tile_pool(name="t", bufs=3))
    epi_pool = ctx.enter_context(tc.tile_pool(name="epi", bufs=2))
    acc_pool = ctx.enter_context(tc.tile_pool(name="acc", bufs=1, space="PSUM"))

    zg_t = consts.tile([NTAP, 128], BF16)
    nc.sync.dma_start(zg_t[:], zg[:])

    # prefetch the first two q super-blocks BEFORE the cen const loads: the
    # first ACT pass gates the whole pipeline, while cen is not needed until
    # the first epilogue a quarter of the way in
    prefetched = {}
    c0 = 0
    for s in range(2):
        w = SUPS[s]
        q_t = q_pool.tile([NTAP, max(SUPS)], BF16)
        nc.sync.dma_start(q_t[:, 0:w], q[:, c0 : c0 + w])
        prefetched[s] = q_t
        c0 += w

    cen_ts = []
    for ci in range(4):
        b0, b1 = CHAIN_START[ci], CHAIN_END[ci]
        cen_c = consts.tile([b1 - b0, V], BF16, tag=f"cen{ci}")
        nc.sync.dma_start(
            cen_c[:], cen[b0 * V : b1 * V].rearrange("(b v) -> b v", v=V)
        )
        cen_ts.append(cen_c)

    # chain ci: num -> bank 2ci (col 1024*ci), den -> bank 2ci+1; block b of
    # the chain accumulates into row b - CHAIN_START[ci]
    acc = acc_pool.tile([128, 4096], F32)

    # PE matmuls only support a single sync-wait: consume zg's DMA semaphore
    # with a throwaway matmul; follow with free-running dummies that keep the
    # PE busy through its p-state ramp while the first DMA/ACT fill runs.
    for _ in range(N_WARMUP):
        nc.tensor.matmul(
            acc[0:64, 0:128], zg_t[:, 0:64], zg_t[:, 0:128],
            start=True, stop=True, skip_group_check=True,
        )

    r_ts = {}

    def epilogue_recip(ci):
        # fires right after the chain's den matmuls stop, overlapping the
        # reciprocal with the same chain's num matmuls
        rows = slice(0, CHAIN_END[ci] - CHAIN_START[ci])
        nco = 1024 * ci
        r_t = epi_pool.tile([21, V], F32, tag="r")
        nc.vector.reciprocal_approx_fast(
            out=r_t[rows, :], in_=acc[rows, nco + 512 : nco + 1024]
        )
        r_ts[ci] = r_t

    def epilogue(ci):
        rows = slice(0, CHAIN_END[ci] - CHAIN_START[ci])
        nco = 1024 * ci
        m_t = epi_pool.tile([21, V], BF16, tag="m")
        nc.vector.tensor_mul(m_t[rows, :], acc[rows, nco : nco + 512], r_ts[ci][rows, :])
        o_t = epi_pool.tile([21, V], BF16, tag="o")
        nc.vector.tensor_add(o_t[rows, :], m_t[rows, :], cen_ts[ci][rows, :])
        nc.sync.dma_start(
            outp[CHAIN_START[ci] * V : CHAIN_END[ci] * V].rearrange(
                "(b v) -> b v", v=V
            ),
            o_t[rows, :],
        )

    def rep_body(first: bool = False):
        c0 = 0
        for s, w in enumerate(SUPS):
            if first and s in prefetched:
                q_t = prefetched[s]
            else:
                q_t = q_pool.tile([NTAP, max(SUPS)], BF16)
                nc.sync.dma_start(q_t[:, 0:w], q[:, c0 : c0 + w])
            e_t = e_pool.tile([NTAP, max(SUPS)], BF16)
            nc.scalar.activation(e_t[:, 0:w], q_t[:, 0:w], DERF, scale=SQA)
            t_t = t_pool.tile([NTAP, max(SUPS)], BF16)
            nc.vector.tensor_mul(t_t[:, 0:w], e_t[:, 0:w], q_t[:, 0:w])

            blk0 = c0 // V
            # den matmuls first (they only need the ACT output), then num
            for src_t, bank_off in ((e_t, 512), (t_t, 0)):
                for bi in range(w // V):
                    b = blk0 + bi
                    ci = _chain_of(b)
                    row = b - CHAIN_START[ci]
                    nco = 1024 * ci + bank_off
                    nc.tensor.matmul(
                        acc[0:64, nco : nco + 512],
                        zg_t[:, 64 - row : 128 - row],
                        src_t[:, bi * V : (bi + 1) * V],
                        start=b == CHAIN_START[ci],
                        stop=b == CHAIN_END[ci] - 1,
                    )
                    if b == CHAIN_END[ci] - 1:
                        if bank_off == 512:
                            epilogue_recip(ci)
                        else:
                            epilogue(ci)
            c0 += w

    if loop > 1:
        with tc.For_i(0, loop):
            rep_body()
    else:
        for _rep in range(repeat):
            rep_body(first=_rep == 0)


def build_program(repeat: int = 1, loop: int = 1) -> bass.Bass:
    nc = bacc.Bacc("TRN2", target_bir_lowering=False, debug=False)
    q = nc.declare_dram_parameter("q", [NTAP, VOX], BF16, isOutput=False)
    cen = nc.declare_dram_parameter("cen", [VOX], BF16, isOutput=False)
    zg = nc.declare_dram_parameter("zg", [NTAP, 128], BF16, isOutput=False)
    outp = nc.declare_dram_parameter("out", [VOX], BF16, isOutput=True)
    with tile.TileContext(nc) as tc, ExitStack() as ctx:
        _kernel_body(ctx, tc, q, cen, zg, outp, repeat=repeat, loop=loop)
    nc.compile()
    return nc


def build_host_inputs(x: np.ndarray) -> list[dict[str, np.ndarray]]:
    """x: [1, 3, 48, 48, 48] float32 -> per-core in_maps."""
    x = np.asarray(x).reshape(C_, D_, H_, W_).astype(np.float32)
    xp = np.pad(x, ((0, 0), (2, 2), (2, 2), (2, 2)), mode="edge")  # [3,52,52,52]
    zg = np.zeros((NTAP, 128), dtype=np.float32)
    zg[:, 64] = _gauss()
    zg = zg.astype(np.float16)
    in_maps = []
    for m in range(NCORES):
        qm = np.empty((NTAP, VOX), dtype=np.float32)
        cenm = np.empty(VOX, dtype=np.float32)
        for o in range(PPC):
            p = m * PPC + o
            c, d = divmod(p, D_)
            center = x[c, d]  # [48, 48]
            cenm[o * PLANE_V : (o + 1) * PLANE_V] = center.reshape(-1)
            for i in range(KS):
                win = np.lib.stride_tricks.sliding_window_view(
                    xp[c, d + i], (H_, W_)
                )  # [5, 5, 48, 48]
                qm[i * 25 : (i + 1) * 25, o * PLANE_V : (o + 1) * PLANE_V] = (
                    win - center
                ).reshape(25, PLANE_V)
        in_maps.append(
            {
                "q": qm.astype(np.float16),
                "cen": cenm.astype(np.float16),
                "zg": zg,
            }
        )
    return in_maps


_PROGRAM: bass.Bass | None = None


def _get_program() -> bass.Bass:
    global _PROGRAM
    if _PROGRAM is None:
        _PROGRAM = build_program()
    return _PROGRAM


def kernel(x: np.ndarray) -> np.ndarray:
    nc = _get_program()
    in_maps = build_host_inputs(x)
    res = run_bass_kernel_spmd(nc, in_maps, list(range(NCORES)))
    planes = np.concatenate(
        [
            np.asarray(res.results[m]["out"]).astype(np.float32).reshape(PPC, H_, W_)
            for m in range(NCORES)
        ],
        axis=0,
    )  # [144, 48, 48]
    return planes.reshape(1, C_, D_, H_, W_).astype(np.float32)


# revision 11
# speedup vs baseline: 7.8707x; 3.9585x over previous
"""Trainium2 Bass kernel: 3D bilateral filter (5x5x5, replicate pad).

Reference math (the wd normalization cancels in the final ratio):
    out(v) = sum_k g_k e_k p_k / sum_k g_k e_k,   e_k = exp(-a (p_k - c)^2)
with a = 1/(2*0.8^2), g the separable 5x5x5 gaussian, p_k the 125
replicate-padded shifted neighbours and c the center voxel.

Substituting q_k = p_k - c gives out = c + (sum g e q) / (sum g e), which
needs only the center-subtracted patches on device.  e_k is computed in a
single ACT pass as Derivative_Erf(sqrt(a) q) = 2/sqrt(pi) exp(-a q^2) -- the
2/sqrt(pi) factor is common to numerator and denominator and cancels.

Device strategy (per core, data-parallel over 144 (c,d) planes, 18 each):
  - host ships Q[125, 41472] bf16 (im2col minus center), cen bf16, and the
    gaussian one-hot lhsT zg (g_k at column 64)
  - per ~3072-col super-block: one im2col DMA, one ACT pass E = derf(sqrt(a) Q)
    and one DVE pass T = E*Q (both bf16; TT runs in 2x_1p mode)
  - PE reduces taps via one-hot matmuls: 512-voxel block b -> one PSUM row
    (num and den in separate banks); g_k rides the lhsT so no bias pass
  - 81 blocks split into 4 accumulation chains on DISJOINT bank pairs
    (chain i: num bank 2i, den bank 2i+1), so each chain's epilogue overlaps
    the later chains' matmuls with no write-after-read PE stall
  - epilogue per chain: r = 1/den (fast approx), o = bf16(num*r) + cen, DMA out
  - warm-up dummy matmuls keep the PE p-state ramping during pipeline fill

The PE never produces an input of the per-block pipeline (the baseline's
center-broadcast made the in-order PE both head and tail of every block's
dependency chain, serializing the whole kernel).
"""

import math
from contextlib import ExitStack

import numpy as np

import concourse.bass as bass
import concourse.mybir as mybir
import concourse.tile as tile
from concourse import bacc
from concourse.bass_utils import run_bass_kernel_spmd

F32 = mybir.dt.float32
BF16 = mybir.dt.float16  # fp16: same 2-byte engine speed as bf16, 8x finer mantissa
DERF = mybir.ActivationFunctionType.Derivative_Erf

SIGMA = 0.8
A_ = 1.0 / (2.0 * SIGMA * SIGMA)  # 0.78125
SQA = math.sqrt(A_)
KS = 5
NTAP = KS * KS * KS  # 125
NCORES = 8
C_, D_, H_, W_ = 3, 48, 48, 48
PPC = (C_ * D_) // NCORES  # 18 planes per core
PLANE_V = H_ * W_  # 2304
VOX = PPC * PLANE_V  # 41472 voxels per core
V = 512  # PSUM block: one bank num + one bank den
NBLK = VOX // V  # 81
CHAIN_START = (0, 21, 42, 63)  # block ranges of the 4 chains
CHAIN_END = (21, 42, 63, NBLK)
# super-blocks: DMA/ACT/DVE instruction granularity; small head for fast fill
SUPS = (1536, 1536, 3072) + (6144,) * 5 + (4608,)
N_WARMUP = 24  # PE p-state warm-up dummies during pipeline fill


def _gauss() -> np.ndarray:
    """The normalized separable 5x5x5 gaussian, float32 [125]."""
    sig = [0.3 * ((k - 1) * 0.5 - 1.0) + 0.8 for k in (KS, KS, KS)]
    grids = np.meshgrid(*[np.arange(k) for k in (KS, KS, KS)], indexing="ij")
    ker = np.ones((KS, KS, KS), dtype=np.float64)
    for k, s, m in zip((KS, KS, KS), sig, grids):
        mean = (k - 1) / 2.0
        ker = ker * np.exp(-((m - mean) ** 2) / (2.0 * s * s))
    ker = ker / ker.sum()
    return ker.astype(np.float32).reshape(-1)


def _chain_of(b: int) -> int:
    for ci in range(4):
        if b < CHAIN_END[ci]:
            return ci
    raise AssertionError(b)


def _kernel_body(ctx, tc, q, cen, zg, outp, repeat: int = 1, loop: int = 1):
    nc = tc.nc

    consts = ctx.enter_context(tc.tile_pool(name="consts", bufs=1))
    q_pool = ctx.enter_context(tc.tile_pool(name="q", bufs=4))
    e_pool = ctx.enter_context(tc.tile_pool(name="e", bufs=3))
    t_pool = ctx.enter_context(tc.tile_pool(name="t", bufs=3))
    epi_pool = ctx.enter_context(tc.tile_pool(name="epi", bufs=2))
    acc_pool = ctx.enter_context(tc.tile_pool(name="acc", bufs=1, space="PSUM"))

    zg_t = consts.tile([NTAP, 128], BF16)
    nc.sync.dma_start(zg_t[:], zg[:])

    # prefetch the first two q super-blocks BEFORE the cen const loads: the
    # first ACT pass gates the whole pipeline, while cen is not needed until
    # the first epilogue a quarter of the way in
    prefetched = {}
    c0 = 0
    for s in range(2):
        w = SUPS[s]
        q_t = q_pool.tile([NTAP, max(SUPS)], BF16)
        nc.sync.dma_start(q_t[:, 0:w], q[:, c0 : c0 + w])
        prefetched[s] = q_t
        c0 += w

    cen_ts = []
    for ci in range(4):
        b0, b1 = CHAIN_START[ci], CHAIN_END[ci]
        cen_c = consts.tile([b1 - b0, V], BF16, tag=f"cen{ci}")
        nc.sync.dma_start(
            cen_c[:], cen[b0 * V : b1 * V].rearrange("(b v) -> b v", v=V)
        )
        cen_ts.append(cen_c)

    # chain ci: num -> bank 2ci (col 1024*ci), den -> bank 2ci+1; block b of
    # the chain accumulates into row b - CHAIN_START[ci]
    acc = acc_pool.tile([128, 4096], F32)

    # PE matmuls only support a single sync-wait: consume zg's DMA semaphore
    # with a throwaway matmul; follow with free-running dummies that keep the
    # PE busy through its p-state ramp while the first DMA/ACT fill runs.
    for _ in range(N_WARMUP):
        nc.tensor.matmul(
            acc[0:64, 0:128], zg_t[:, 0:64], zg_t[:, 0:128],
            start=True, stop=True, skip_group_check=True,
        )

    r_ts = {}

    def epilogue_recip(ci):
        # fires right after the chain's den matmuls stop, overlapping the
        # reciprocal with the same chain's num matmuls
        rows = slice(0, CHAIN_END[ci] - CHAIN_START[ci])
        nco = 1024 * ci
        r_t = epi_pool.tile([21, V], F32, tag="r")
        nc.vector.reciprocal_approx_fast(
            out=r_t[rows, :], in_=acc[rows, nco + 512 : nco + 1024]
        )
        r_ts[ci] = r_t

    def epilogue(ci):
        rows = slice(0, CHAIN_END[ci] - CHAIN_START[ci])
        nco = 1024 * ci
        m_t = epi_pool.tile([21, V], BF16, tag="m")
        nc.vector.tensor_mul(m_t[rows, :], acc[rows, nco : nco + 512], r_ts[ci][rows, :])
        o_t = epi_pool.tile([21, V], BF16, tag="o")
        nc.vector.tensor_add(o_t[rows, :], m_t[rows, :], cen_ts[ci][rows, :])
        nc.sync.dma_start(
            outp[CHAIN_START[ci] * V : CHAIN_END[ci] * V].rearrange(
                "(b v) -> b v", v=V
            ),
            o_t[rows, :],
        )

    def rep_body(first: bool = False):
        c0 = 0
        for s, w in enumerate(SUPS):
            if first and s in prefetched:
                q_t = prefetched[s]
            else:
                q_t = q_pool.tile([NTAP, max(SUPS)], BF16)
                nc.sync.dma_start(q_t[:, 0:w], q[:, c0 : c0 + w])
            e_t = e_pool.tile([NTAP, max(SUPS)], BF16)
            nc.scalar.activation(e_t[:, 0:w], q_t[:, 0:w], DERF, scale=SQA)
            t_t = t_pool.tile([NTAP, max(SUPS)], BF16)
            nc.vector.tensor_mul(t_t[:, 0:w], e_t[:, 0:w], q_t[:, 0:w])

            blk0 = c0 // V
            # den matmuls first (they only need the ACT output), then num
            for src_t, bank_off in ((e_t, 512), (t_t, 0)):
                for bi in range(w // V):
                    b = blk0 + bi
                    ci = _chain_of(b)
                    row = b - CHAIN_START[ci]
                    nco = 1024 * ci + bank_off
                    nc.tensor.matmul(
                        acc[0:64, nco : nco + 512],
                        zg_t[:, 64 - row : 128 - row],
                        src_t[:, bi * V : (bi + 1) * V],
                        start=b == CHAIN_START[ci],
                        stop=b == CHAIN_END[ci] - 1,
                    )
                    if b == CHAIN_END[ci] - 1:
                        if bank_off == 512:
                            epilogue_recip(ci)
                        else:
                            epilogue(ci)
            c0 += w

    if loop > 1:
        with tc.For_i(0, loop):
            rep_body()
    else:
        for _rep in range(repeat):
            rep_body(first=_rep == 0)


def build_program(repeat: int = 1, loop: int = 1) -> bass.Bass:
    nc = bacc.Bacc("TRN2", target_bir_lowering=False, debug=False)
    q = nc.declare_dram_parameter("q", [NTAP, VOX], BF16, isOutput=False)
    cen = nc.declare_dram_parameter("cen", [VOX], BF16, isOutput=False)
    zg = nc.declare_dram_parameter("zg", [NTAP, 128], BF16, isOutput=False)
    outp = nc.declare_dram_parameter("out", [VOX], BF16, isOutput=True)
    with tile.TileContext(nc) as tc, ExitStack() as ctx:
        _kernel_body(ctx, tc, q, cen, zg, outp, repeat=repeat, loop=loop)
    nc.compile()
    return nc


def build_host_inputs(x: np.ndarray) -> list[dict[str, np.ndarray]]:
    """x: [1, 3, 48, 48, 48] float32 -> per-core in_maps."""
    x = np.asarray(x).reshape(C_, D_, H_, W_).astype(np.float32)
    xp = np.pad(x, ((0, 0), (2, 2), (2, 2), (2, 2)), mode="edge")  # [3,52,52,52]
    zg = np.zeros((NTAP, 128), dtype=np.float32)
    zg[:, 64] = _gauss()
    zg = zg.astype(np.float16)
    in_maps = []
    for m in range(NCORES):
        qm = np.empty((NTAP, VOX), dtype=np.float32)
        cenm = np.empty(VOX, dtype=np.float32)
        for o in range(PPC):
            p = m * PPC + o
            c, d = divmod(p, D_)
            center = x[c, d]  # [48, 48]
            cenm[o * PLANE_V : (o + 1) * PLANE_V] = center.reshape(-1)
            for i in range(KS):
                win = np.lib.stride_tricks.sliding_window_view(
                    xp[c, d + i], (H_, W_)
                )  # [5, 5, 48, 48]
                qm[i * 25 : (i + 1) * 25, o * PLANE_V : (o + 1) * PLANE_V] = (
                    win - center
                ).reshape(25, PLANE_V)
        in_maps.append(
            {
                "q": qm.astype(np.float16),
                "cen": cenm.astype(np.float16),
                "zg": zg,
            }
        )
    return in_maps


_PROGRAM: bass.Bass | None = None


def _get_program() -> bass.Bass:
    global _PROGRAM
    if _PROGRAM is None:
        _PROGRAM = build_program()
    return _PROGRAM


def kernel(x: np.ndarray) -> np.ndarray:
    nc = _get_program()
    in_maps = build_host_inputs(x)
    res = run_bass_kernel_spmd(nc, in_maps, list(range(NCORES)))
    planes = np.concatenate(
        [
            np.asarray(res.results[m]["out"]).astype(np.float32).reshape(PPC, H_, W_)
            for m in range(NCORES)
        ],
        axis=0,
    )  # [144, 48, 48]
    return planes.reshape(1, C_, D_, H_, W_).astype(np.float32)
